# revision 1
# baseline (speedup 1.0000x reference)
"""DeepBilateralNetCurves (HDRNet-style) Trainium2 kernel.

Split of work:
  - Host (numpy): the tiny lowres CNN (256x256 -> 12x8x16x16 bilateral grid,
    ~165 MFLOP on 1.5 MB of input), plus weight folding / layout prep.
  - Device (8 NeuronCores, Bass/Tile): the memory-bound fullres stage
    (guide map -> luma tents -> trilinear grid slice -> per-pixel affine),
    which is ~97% of the memory traffic (2x3x1024x1024 in + out).

Sharding: fullres rows are sharded 8 ways (batch b = core//4, 256 rows per
core); the tiny grid-derived constants are replicated per core.

Device layout ("quadrant layout"): for a core's [256, 1024] slice,
  partition p = xb*8 + yb*2 + hh   (xb: 16 x-blocks of 64 cols,
                                    yb: 4 local y-blocks of 64 rows,
                                    hh: which 32-row half of the y-block)
  free      f = hsub*32 + r        (hsub: row within half-block, r: col within
                                    a 32-col half of the x-block)
and two tile families per tensor: half L (w in [64xb, 64xb+32), fx = xb-1)
and half R (w in [64xb+32, 64xb+64), fx = xb).  In this layout the bilinear
corner cell indices (fy, fx) are constant per partition, so the four grid
corner combinations A, B, C, D (per output channel j and luma bin z) are
per-partition scalars, and the per-pixel trilinear slice becomes
    coeff_j = sum_z [ A*T_z + B*(u*T_z) + C*(v*T_z) + D*(u*v*T_z) ]
with T_z the luma tent weights and u, v fixed free-axis patterns.
"""

import os
import numpy as np

import concourse.bass as bass
import concourse.bacc as bacc
import concourse.mybir as mybir
from concourse.tile import TileContext
from concourse.bass_utils import run_bass_kernel_spmd

F32 = mybir.dt.float32
BF16 = mybir.dt.bfloat16
ALU = mybir.AluOpType

LUMA, GPTS = 8, 16
NIN, NOUT = 3, 3
H, W = 1024, 1024
B = 2
ROWS_PER_CORE = 256
N_CORES = 8


# ---------------------------------------------------------------------------
# Host-side reference CNN (numpy float32, mirrors reference.py exactly)
# ---------------------------------------------------------------------------

def _conv(x, w, b=None, stride=1, relu=True):
    # x: [C, H, W]; w: [O, I, k, k]; cross-correlation, pad k//2
    k = w.shape[2]
    p = k // 2
    if p:
        xp = np.pad(x, ((0, 0), (p, p), (p, p)))
    else:
        xp = x
    Hs, Ws = xp.shape[1], xp.shape[2]
    Ho = (Hs - k) // stride + 1
    Wo = (Ws - k) // stride + 1
    win = np.lib.stride_tricks.sliding_window_view(xp, (k, k), axis=(1, 2))
    win = win[:, ::stride, ::stride]           # [I, Ho, Wo, k, k]
    y = np.einsum("ihwkl,oikl->ohw", win, w, optimize=True).astype(np.float32)
    if b is not None:
        y = y + b[:, None, None]
    return np.maximum(y, 0.0) if relu else y


def _grid_from_lowres(inp):
    """Returns grid [B, 12, LUMA, 16, 16] float32."""
    lows = np.asarray(inp["image_lowres"], np.float32)
    grids = []
    for bi in range(lows.shape[0]):
        x = lows[bi]
        x = _conv(x, inp["sw0"], inp["sb0"], 2)
        x = _conv(x, inp["sw1"], inp["sb1"], 2)
        x = _conv(x, inp["sw2"], inp["sb2"], 2)
        x = _conv(x, inp["sw3"], inp["sb3"], 2)          # [64,16,16]
        g = _conv(x, inp["gw0"], inp["gb0"], 2)
        g = _conv(g, inp["gw1"], inp["gb1"], 2)          # [64,4,4]
        g = g.reshape(-1)                                # [1024]
        g = np.maximum(g @ inp["fw0"].T + inp["fb0"], 0)
        g = np.maximum(g @ inp["fw1"].T + inp["fb1"], 0)
        g = g @ inp["fw2"].T + inp["fb2"]                # [64]
        loc = _conv(x, inp["lw0"], inp["lb0"], 1)
        loc = _conv(loc, inp["lw1"], None, 1, relu=False)
        fusion = np.maximum(g[:, None, None] + loc, 0)   # [64,16,16]
        co = _conv(fusion, inp["pw"], inp["pb"], 1, relu=False)  # [96,16,16]
        grid = co.reshape(LUMA, NOUT * (NIN + 1), 16, 16).transpose(1, 0, 2, 3)
        grids.append(grid.astype(np.float32))
    return np.stack(grids)                               # [B,12,8,16,16]


def _guide_linear_params(inp):
    """The guide map here is linear in rgb: verify & fold.

    guide g = clip(sum_c projw_c * pwl_c(ccm(rgb)_c) + proj_b, 0, 1),
    pwl_c(y) = sum_k slopes_ck * relu(y - shifts_ck).
    When only slope k=0 is nonzero with shift 0, and ccm output is provably
    >= 0 on [0,1]^3, pwl is linear -> g = w . rgb + beta.
    Device then computes gz = clamp(8*g - 0.5, 0, 7) (equivalent to the
    reference's clip-then-scale followed by clipped-tap accumulation).
    """
    slopes = np.asarray(inp["slopes"], np.float32).reshape(NIN, GPTS)
    shifts = np.asarray(inp["shifts"], np.float32).reshape(NIN, GPTS)
    M = np.asarray(inp["ccm_w"], np.float32).reshape(NIN, NIN)
    bc = np.asarray(inp["ccm_b"], np.float32)
    pw = np.asarray(inp["proj_w"], np.float32).reshape(NIN)
    pb = float(np.asarray(inp["proj_b"], np.float32).reshape(-1)[0])
    if not (np.all(slopes[:, 1:] == 0) and np.all(shifts[:, 0] == 0)):
        raise NotImplementedError("general piecewise-linear guide not folded")
    ymin = bc + np.minimum(M, 0).sum(axis=1)
    if not np.all(ymin >= 0):
        raise NotImplementedError("ccm output can go negative; relu not linear")
    s0 = slopes[:, 0]                                    # per-channel slope
    w = np.einsum("c,c,ci->i", pw, s0, M)
    beta = float(np.dot(pw * s0, bc) + pb)
    # fold gz = 8*g - 0.5
    return (w * 8.0).astype(np.float32), beta * 8.0 - 0.5


# ---------------------------------------------------------------------------
# Host-side layout helpers
# ---------------------------------------------------------------------------

def _quadrantize(x):
    """[C, 256, 1024] -> [C, 2(half), 128(p), 1024(f)] in quadrant layout."""
    C = x.shape[0]
    v = x.reshape(C, 4, 2, 32, 16, 2, 32)       # c, yb, hh, hsub, xb, half, r
    v = v.transpose(0, 5, 4, 1, 2, 3, 6)        # c, half, xb, yb, hh, hsub, r
    return np.ascontiguousarray(v.reshape(C, 2, 128, 1024), np.float32)


def _unquadrantize(x):
    """[C, 2, 128, 1024] -> [C, 256, 1024]."""
    C = x.shape[0]
    v = x.reshape(C, 2, 16, 4, 2, 32, 32)       # c, half, xb, yb, hh, hsub, r
    v = v.transpose(0, 3, 4, 5, 2, 1, 6)        # c, yb, hh, hsub, xb, half, r
    return np.ascontiguousarray(v.reshape(C, 256, 1024), np.float32)


def _build_vec(grid_b, h0):
    """Per-partition corner combos: [128, 2*96*4] f32.

    index = half*384 + (j*8+z)*4 + field,  field in (A, B, C, D).
    """
    vec = np.zeros((128, 768), np.float32)
    for p in range(128):
        xb, rem = divmod(p, 8)
        yb, hh = divmod(rem, 2)
        hb = h0 + yb * 64 + hh * 32
        k = hb // 32
        fy = (k - 1) // 2
        cy0 = min(max(fy, 0), 15)
        cy1 = min(max(fy + 1, 0), 15)
        for half in range(2):
            fx = xb - 1 if half == 0 else xb
            cx0 = min(max(fx, 0), 15)
            cx1 = min(max(fx + 1, 0), 15)
            g00 = grid_b[:, :, cy0, cx0]        # [12, 8]
            g01 = grid_b[:, :, cy0, cx1]
            g10 = grid_b[:, :, cy1, cx0]
            g11 = grid_b[:, :, cy1, cx1]
            A = g00
            Bf = g01 - g00
            Cf = g10 - g00
            Df = g11 - g01 - g10 + g00
            blk = np.stack([A, Bf, Cf, Df], axis=-1)    # [12, 8, 4]
            vec[p, half * 384:(half + 1) * 384] = blk.reshape(-1)
    return vec


def _uv_planes():
    """U planes per half and V plane, [128, 1024] f32 each."""
    r = np.arange(32, dtype=np.float32)
    hsub = np.arange(32, dtype=np.float32)
    uL = (r + 0.5) / 64.0 + 0.5                 # half L
    uR = (r + 0.5) / 64.0                       # half R
    U = np.zeros((2, 128, 1024), np.float32)
    U[0] = np.tile(uL[None, :], (128, 32)).reshape(128, 1024)
    U[1] = np.tile(uR[None, :], (128, 32)).reshape(128, 1024)
    V = np.zeros((128, 1024), np.float32)
    vbase = (hsub + 0.5) / 64.0                 # [32]
    vplane_hh = np.repeat(vbase, 32)[None, :]   # [1, 1024] (hsub major)
    for p in range(128):
        hh = p % 2
        V[p] = vplane_hh + (0.5 if hh == 0 else 0.0)
    return U, V


# ---------------------------------------------------------------------------
# Device program
# ---------------------------------------------------------------------------

_PROGRAM_CACHE = {}


def _build_program(w_guide, beta):
    key = (tuple(np.round(w_guide, 10)), round(beta, 10))
    if key in _PROGRAM_CACHE:
        return _PROGRAM_CACHE[key]

    nc = bacc.Bacc("TRN2", target_bir_lowering=False)
    RGB = nc.dram_tensor("rgbq", [3, 2, 128, 1024], F32, kind="ExternalInput")
    VEC = nc.dram_tensor("vec", [128, 768], F32, kind="ExternalInput")
    UPL = nc.dram_tensor("uplanes", [2, 128, 1024], F32, kind="ExternalInput")
    VPL = nc.dram_tensor("vplane", [128, 1024], F32, kind="ExternalInput")
    OUT = nc.dram_tensor("outq", [3, 2, 128, 1024], F32, kind="ExternalOutput")

    w0, w1, w2 = (float(x) for x in w_guide)

    CH = 512  # free-dim chunk

    with TileContext(nc) as tc:
        with tc.tile_pool(name="const", bufs=1) as cpool, \
             tc.tile_pool(name="io", bufs=1) as iopool, \
             tc.tile_pool(name="fam", bufs=1) as fpool, \
             tc.tile_pool(name="work", bufs=1) as wpool:

            vec_t = cpool.tile([128, 768], F32, tag="vec")
            nc.sync.dma_start(vec_t[:], VEC[:])
            vpl_t = cpool.tile([128, 1024], F32, tag="vpl")
            nc.sync.dma_start(vpl_t[:], VPL[:])
            # Touch DMA'd tensors with plain copies so semaphore waits land
            # on TENSOR_COPY (ptr-scalar ISA structs have few wait slots).
            touch = cpool.tile([128, 1], F32, tag="touch")
            nc.vector.tensor_copy(touch[:], vec_t[:, 0:1])
            touchb = cpool.tile([128, 1], F32, tag="touchb")
            nc.vector.tensor_copy(touchb[:], vpl_t[:, 0:1])

            for half in range(2):
                rgb_t = []
                for c in range(3):
                    t = iopool.tile([128, 1024], F32, tag=f"rgb{c}")
                    nc.sync.dma_start(t[:], RGB[c, half])
                    rgb_t.append(t)
                upl_t = iopool.tile([128, 1024], F32, tag="upl")
                nc.sync.dma_start(upl_t[:], UPL[half])

                for ci in range(1024 // CH):
                    sl = slice(ci * CH, (ci + 1) * CH)
                    # guide: gz = clamp(w.rgb + beta, 0, 7) (8x, -0.5 folded)
                    gz = wpool.tile([128, CH], F32, tag="gz")
                    tg = wpool.tile([128, CH], F32, tag="tg")
                    nc.vector.tensor_scalar(gz[:], rgb_t[0][:, sl], w0, beta,
                                            ALU.mult, ALU.add)
                    nc.vector.tensor_scalar(tg[:], rgb_t[1][:, sl], w1, None,
                                            ALU.mult)
                    nc.vector.tensor_tensor(gz[:], gz[:], tg[:], ALU.add)
                    nc.vector.tensor_scalar(tg[:], rgb_t[2][:, sl], w2, None,
                                            ALU.mult)
                    nc.vector.tensor_tensor(gz[:], gz[:], tg[:], ALU.add)
                    nc.vector.tensor_scalar(gz[:], gz[:], 0.0, 7.0,
                                            ALU.max, ALU.min)
                    neg = wpool.tile([128, CH], F32, tag="neg")
                    nc.vector.tensor_scalar(neg[:], gz[:], -1.0, None,
                                            ALU.mult)

                    # tents T_z = relu(min(gz - z + 1, z + 1 - gz)) + families
                    fams = []   # fams[z] = (t, ut, vt, uvt)
                    for z in range(LUMA):
                        m = wpool.tile([128, CH], F32, tag="scratch")
                        nc.vector.scalar_tensor_tensor(
                            m[:], gz[:], float(-2 * z), neg[:],
                            ALU.add, ALU.min)
                        t = fpool.tile([128, CH], F32, tag=f"t{z}")
                        nc.vector.tensor_scalar(t[:], m[:], float(z + 1), 0.0,
                                                ALU.add, ALU.max)
                        ut = fpool.tile([128, CH], F32, tag=f"ut{z}")
                        nc.vector.tensor_tensor(ut[:], t[:], upl_t[:, sl],
                                                ALU.mult)
                        vt = fpool.tile([128, CH], F32, tag=f"vt{z}")
                        nc.vector.tensor_tensor(vt[:], t[:], vpl_t[:, sl],
                                                ALU.mult)
                        uvt = fpool.tile([128, CH], F32, tag=f"uvt{z}")
                        nc.vector.tensor_tensor(uvt[:], ut[:], vpl_t[:, sl],
                                                ALU.mult)
                        fams.append((t, ut, vt, uvt))

                    # contraction + affine accumulation
                    outacc = [wpool.tile([128, CH], F32, tag=f"oacc{o}",
                                         name=f"oacc{o}")
                              for o in range(NOUT)]
                    coeff = wpool.tile([128, CH], F32, tag="coeff")

                    facc = [wpool.tile([128, CH], F32, tag=f"facc{f}",
                                       name=f"facc{f}") for f in range(4)]
                    for j in range(12):
                        o, i = divmod(j, 4)
                        for f in range(4):
                            for z in range(LUMA):
                                base = half * 384 + (j * 8 + z) * 4
                                sc = vec_t[:, base + f:base + f + 1]
                                fam = fams[z][f]
                                if z == 0:
                                    nc.vector.tensor_scalar(
                                        facc[f][:], fam[:], sc, None, ALU.mult)
                                else:
                                    nc.vector.scalar_tensor_tensor(
                                        facc[f][:], fam[:], sc, facc[f][:],
                                        ALU.mult, ALU.add)
                        nc.vector.tensor_tensor(facc[0][:], facc[0][:],
                                                facc[1][:], ALU.add)
                        nc.vector.tensor_tensor(facc[2][:], facc[2][:],
                                                facc[3][:], ALU.add)
                        nc.vector.tensor_tensor(coeff[:], facc[0][:],
                                                facc[2][:], ALU.add)
                        if i < 3:
                            nc.vector.tensor_tensor(coeff[:], coeff[:],
                                                    rgb_t[i][:, sl], ALU.mult)
                        if i == 0:
                            nc.vector.tensor_copy(outacc[o][:], coeff[:])
                        else:
                            nc.vector.tensor_tensor(outacc[o][:],
                                                    outacc[o][:], coeff[:],
                                                    ALU.add)

                    for o in range(NOUT):
                        res = iopool.tile([128, CH], F32, tag=f"res{o}")
                        nc.vector.tensor_scalar(res[:], outacc[o][:],
                                                0.0, 1.0, ALU.max, ALU.min)
                        nc.sync.dma_start(OUT[o, half, :, sl], res[:])

    nc.finalize()
    _PROGRAM_CACHE[key] = nc
    return nc


# ---------------------------------------------------------------------------
# Entry point
# ---------------------------------------------------------------------------

def kernel(**inputs):
    fullres = np.asarray(inputs["image_fullres"], np.float32)
    grid = _grid_from_lowres(inputs)                     # [B,12,8,16,16]
    w_guide, beta = _guide_linear_params(inputs)
    U, V = _uv_planes()

    nc = _build_program(w_guide, beta)

    in_maps = []
    for core in range(N_CORES):
        bi = core // 4
        h0 = ROWS_PER_CORE * (core % 4)
        rgbq = _quadrantize(fullres[bi, :, h0:h0 + ROWS_PER_CORE, :])
        in_maps.append({
            "rgbq": rgbq,
            "vec": _build_vec(grid[bi], h0),
            "uplanes": U,
            "vplane": V,
        })

    trace = os.environ.get("KERNEL_TRACE", "0") == "1"
    try:
        res = run_bass_kernel_spmd(nc, in_maps, core_ids=list(range(N_CORES)),
                                   trace=trace)
    except ModuleNotFoundError:
        # NTFF profiling hooks unavailable in this container
        res = run_bass_kernel_spmd(nc, in_maps, core_ids=list(range(N_CORES)),
                                   trace=False)
    if trace and res.exec_time_ns is not None:
        print(f"HW exec time: {res.exec_time_ns} ns")

    out = np.zeros((B, 3, H, W), np.float32)
    for core in range(N_CORES):
        bi = core // 4
        h0 = ROWS_PER_CORE * (core % 4)
        out[bi, :, h0:h0 + ROWS_PER_CORE, :] = _unquadrantize(
            res.results[core]["outq"])
    return out



# revision 2
# speedup vs baseline: 5.6267x; 5.6267x over previous
"""DeepBilateralNetCurves (HDRNet-style) Trainium2 kernel.

Split of work:
  - Host (numpy): the tiny lowres CNN (256x256 -> 12x8x16x16 bilateral grid,
    ~165 MFLOP on 1.5 MB of input), plus weight folding / layout prep.
  - Device (8 NeuronCores, Bass/Tile): the memory-bound fullres stage
    (guide map -> luma tents -> trilinear grid slice -> per-pixel affine),
    which is ~97% of the memory traffic (2x3x1024x1024 in + out).

Sharding: fullres rows are sharded 8 ways (batch b = core//4, 256 rows per
core); the tiny grid-derived constants are replicated per core.

Device layout ("quadrant layout"): for a core's [256, 1024] slice,
  partition p = xb*8 + yb*2 + hh   (xb: 16 x-blocks of 64 cols,
                                    yb: 4 local y-blocks of 64 rows,
                                    hh: which 32-row half of the y-block)
  free      f = hsub*32 + r        (hsub: row within half-block, r: col within
                                    a 32-col half of the x-block)
and two tile families per tensor: half L (w in [64xb, 64xb+32), fx = xb-1)
and half R (w in [64xb+32, 64xb+64), fx = xb).  In this layout the bilinear
corner cell indices (fy, fx) are constant per partition, so the four grid
corner combinations A, B, C, D (per output channel j and luma bin z) are
per-partition scalars, and the per-pixel trilinear slice becomes
    coeff_j = sum_z [ A*T_z + B*(u*T_z) + C*(v*T_z) + D*(u*v*T_z) ]
with T_z the luma tent weights and u, v fixed free-axis patterns.

Wall-clock structure: the axon tunnel to the remote NeuronCores has ~85 ms
per-transfer latency, ~190 MB/s up and ~70 MB/s down, and the jax/bass2jax
glue re-traces and re-compiles on every run_bass_kernel_spmd call.  So the
runner here (a) builds + jits the shard_map executable once and caches it,
(b) keeps the constant u/v planes device-resident, (c) ships no output
donation buffers, and (d) returns the output as fp16 (the |rel| tolerance
budget allows ~5e-4; halves the slow downlink).
"""

import numpy as np

import jax

import concourse.bass as bass  # noqa: F401  (keeps bass registered)
import concourse.bacc as bacc
import concourse.bass2jax as b2j
import concourse.mybir as mybir
from concourse.tile import TileContext
from jax.experimental.shard_map import shard_map
from jax.sharding import Mesh, NamedSharding, PartitionSpec

F32 = mybir.dt.float32
F16 = mybir.dt.float16
ALU = mybir.AluOpType

LUMA, GPTS = 8, 16
NIN, NOUT = 3, 3
H, W = 1024, 1024
B = 2
ROWS_PER_CORE = 256
N_CORES = 8


# ---------------------------------------------------------------------------
# Host-side reference CNN (numpy float32, mirrors reference.py exactly)
# ---------------------------------------------------------------------------

def _conv(x, w, b=None, stride=1, relu=True):
    # x: [C, H, W]; w: [O, I, k, k]; cross-correlation, pad k//2
    k = w.shape[2]
    p = k // 2
    if p:
        xp = np.pad(x, ((0, 0), (p, p), (p, p)))
    else:
        xp = x
    win = np.lib.stride_tricks.sliding_window_view(xp, (k, k), axis=(1, 2))
    win = win[:, ::stride, ::stride]           # [I, Ho, Wo, k, k]
    y = np.einsum("ihwkl,oikl->ohw", win, w, optimize=True).astype(np.float32)
    if b is not None:
        y = y + b[:, None, None]
    return np.maximum(y, 0.0) if relu else y


def _grid_from_lowres(inp):
    """Returns grid [B, 12, LUMA, 16, 16] float32."""
    lows = np.asarray(inp["image_lowres"], np.float32)
    grids = []
    for bi in range(lows.shape[0]):
        x = lows[bi]
        x = _conv(x, inp["sw0"], inp["sb0"], 2)
        x = _conv(x, inp["sw1"], inp["sb1"], 2)
        x = _conv(x, inp["sw2"], inp["sb2"], 2)
        x = _conv(x, inp["sw3"], inp["sb3"], 2)          # [64,16,16]
        g = _conv(x, inp["gw0"], inp["gb0"], 2)
        g = _conv(g, inp["gw1"], inp["gb1"], 2)          # [64,4,4]
        g = g.reshape(-1)                                # [1024]
        g = np.maximum(g @ inp["fw0"].T + inp["fb0"], 0)
        g = np.maximum(g @ inp["fw1"].T + inp["fb1"], 0)
        g = g @ inp["fw2"].T + inp["fb2"]                # [64]
        loc = _conv(x, inp["lw0"], inp["lb0"], 1)
        loc = _conv(loc, inp["lw1"], None, 1, relu=False)
        fusion = np.maximum(g[:, None, None] + loc, 0)   # [64,16,16]
        co = _conv(fusion, inp["pw"], inp["pb"], 1, relu=False)  # [96,16,16]
        grid = co.reshape(LUMA, NOUT * (NIN + 1), 16, 16).transpose(1, 0, 2, 3)
        grids.append(grid.astype(np.float32))
    return np.stack(grids)                               # [B,12,8,16,16]


def _guide_linear_params(inp):
    """The guide map here is linear in rgb: verify & fold.

    guide g = clip(sum_c projw_c * pwl_c(ccm(rgb)_c) + proj_b, 0, 1),
    pwl_c(y) = sum_k slopes_ck * relu(y - shifts_ck).
    When only slope k=0 is nonzero with shift 0, and ccm output is provably
    >= 0 on [0,1]^3, pwl is linear -> g = w . rgb + beta.
    Device then computes gz = clamp(8*g - 0.5, 0, 7) (equivalent to the
    reference's clip-then-scale followed by clipped-tap accumulation).
    """
    slopes = np.asarray(inp["slopes"], np.float32).reshape(NIN, GPTS)
    shifts = np.asarray(inp["shifts"], np.float32).reshape(NIN, GPTS)
    M = np.asarray(inp["ccm_w"], np.float32).reshape(NIN, NIN)
    bc = np.asarray(inp["ccm_b"], np.float32)
    pw = np.asarray(inp["proj_w"], np.float32).reshape(NIN)
    pb = float(np.asarray(inp["proj_b"], np.float32).reshape(-1)[0])
    if not (np.all(slopes[:, 1:] == 0) and np.all(shifts[:, 0] == 0)):
        raise NotImplementedError("general piecewise-linear guide not folded")
    ymin = bc + np.minimum(M, 0).sum(axis=1)
    if not np.all(ymin >= 0):
        raise NotImplementedError("ccm output can go negative; relu not linear")
    s0 = slopes[:, 0]                                    # per-channel slope
    w = np.einsum("c,c,ci->i", pw, s0, M)
    beta = float(np.dot(pw * s0, bc) + pb)
    # fold gz = 8*g - 0.5
    return (w * 8.0).astype(np.float32), beta * 8.0 - 0.5


# ---------------------------------------------------------------------------
# Host-side layout helpers (all vectorized over the 8 cores)
# ---------------------------------------------------------------------------

def _quadrantize_all(fullres):
    """[B,3,1024,1024] -> [24, 2, 128, 1024]: concat over cores of the
    per-core [3, 2(half), 128(p), 1024(f)] quadrant-layout tensor,
    core = bi*4 + hblk."""
    v = fullres.reshape(B, 3, 4, 4, 2, 32, 16, 2, 32)
    #                   bi c  hblk yb hh hsub xb half r
    v = v.transpose(0, 2, 1, 7, 6, 3, 4, 5, 8)
    #               bi hblk c half xb yb hh hsub r
    return np.ascontiguousarray(v.reshape(24, 2, 128, 1024))


def _unquadrantize_all(res):
    """[24, 2, 128, 1024] (fp16) -> [B, 3, 1024, 1024] float32."""
    v = res.reshape(B, 4, 3, 2, 16, 4, 2, 32, 32)
    #               bi hblk c half xb yb hh hsub r
    v = v.transpose(0, 2, 1, 5, 6, 7, 4, 3, 8)
    #               bi c hblk yb hh hsub xb half r
    return v.reshape(B, 3, 1024, 1024).astype(np.float32)


def _build_vec_all(grid):
    """Per-partition corner combos for all 8 cores: [8, 128, 768] f32.

    vec[core, p, half*384 + (j*8+z)*4 + field], field in (A, B, C, D).
    """
    hblk = np.arange(4)
    yb = np.arange(4)
    hh = np.arange(2)
    # k = hb//32 for hb = 256*hblk + 64*yb + 32*hh
    k = 8 * hblk[:, None, None] + 2 * yb[None, :, None] + hh[None, None, :]
    fy = (k - 1) // 2                                    # [4,4,2]
    cy0 = np.clip(fy, 0, 15)
    cy1 = np.clip(fy + 1, 0, 15)
    xb = np.arange(16)
    half = np.arange(2)
    fx = xb[:, None] - 1 + half[None, :]                 # [16,2]
    cx0 = np.clip(fx, 0, 15)
    cx1 = np.clip(fx + 1, 0, 15)

    def g(cy, cx):
        # grid [B,12,8,16,16] indexed at [.., cy(4,4,2)broadcast, cx(16,2)]
        cyE = cy[:, :, :, None, None]                    # [4,4,2,1,1]
        cxE = cx[None, None, None, :, :]                 # [1,1,1,16,2]
        return grid[:, :, :, cyE, cxE]                   # [B,12,8,4,4,2,16,2]

    g00, g01, g10, g11 = g(cy0, cx0), g(cy0, cx1), g(cy1, cx0), g(cy1, cx1)
    F = np.stack([g00, g01 - g00, g10 - g00, g11 - g01 - g10 + g00], axis=-1)
    # F: [bi, j, z, hblk, yb, hh, xb, half, field]
    F = F.transpose(0, 3, 6, 4, 5, 7, 1, 2, 8)
    #               bi hblk xb yb hh half j z field
    return np.ascontiguousarray(F.reshape(8, 128, 768), np.float32)


def _uv_planes():
    """U planes per half and V plane, [128, 1024] f32 each."""
    r = np.arange(32, dtype=np.float32)
    hsub = np.arange(32, dtype=np.float32)
    uL = (r + 0.5) / 64.0 + 0.5                 # half L
    uR = (r + 0.5) / 64.0                       # half R
    U = np.zeros((2, 128, 1024), np.float32)
    U[0] = np.tile(uL[None, :], (128, 32)).reshape(128, 1024)
    U[1] = np.tile(uR[None, :], (128, 32)).reshape(128, 1024)
    V = np.zeros((128, 1024), np.float32)
    vbase = (hsub + 0.5) / 64.0                 # [32]
    vplane_hh = np.repeat(vbase, 32)[None, :]   # [1, 1024] (hsub major)
    for p in range(128):
        hh = p % 2
        V[p] = vplane_hh + (0.5 if hh == 0 else 0.0)
    return U, V


# ---------------------------------------------------------------------------
# Device program
# ---------------------------------------------------------------------------

def _build_program(w_guide, beta):
    nc = bacc.Bacc("TRN2", target_bir_lowering=False)
    RGB = nc.dram_tensor("rgbq", [3, 2, 128, 1024], F32, kind="ExternalInput")
    VEC = nc.dram_tensor("vec", [128, 768], F32, kind="ExternalInput")
    UPL = nc.dram_tensor("uplanes", [2, 128, 1024], F32, kind="ExternalInput")
    VPL = nc.dram_tensor("vplane", [128, 1024], F32, kind="ExternalInput")
    OUT = nc.dram_tensor("outq", [3, 2, 128, 1024], F16, kind="ExternalOutput")

    w0, w1, w2 = (float(x) for x in w_guide)

    CH = 512  # free-dim chunk

    with TileContext(nc) as tc:
        with tc.tile_pool(name="const", bufs=1) as cpool, \
             tc.tile_pool(name="io", bufs=1) as iopool, \
             tc.tile_pool(name="fam", bufs=1) as fpool, \
             tc.tile_pool(name="work", bufs=1) as wpool:

            vec_t = cpool.tile([128, 768], F32, tag="vec")
            nc.sync.dma_start(vec_t[:], VEC[:])
            vpl_t = cpool.tile([128, 1024], F32, tag="vpl")
            nc.sync.dma_start(vpl_t[:], VPL[:])
            # Touch DMA'd tensors with plain copies so semaphore waits land
            # on TENSOR_COPY (ptr-scalar ISA structs have few wait slots).
            touch = cpool.tile([128, 1], F32, tag="touch")
            nc.vector.tensor_copy(touch[:], vec_t[:, 0:1])
            touchb = cpool.tile([128, 1], F32, tag="touchb")
            nc.vector.tensor_copy(touchb[:], vpl_t[:, 0:1])

            for half in range(2):
                rgb_t = []
                for c in range(3):
                    t = iopool.tile([128, 1024], F32, tag=f"rgb{c}")
                    nc.sync.dma_start(t[:], RGB[c, half])
                    rgb_t.append(t)
                upl_t = iopool.tile([128, 1024], F32, tag="upl")
                nc.sync.dma_start(upl_t[:], UPL[half])

                for ci in range(1024 // CH):
                    sl = slice(ci * CH, (ci + 1) * CH)
                    # guide: gz = clamp(w.rgb + beta, 0, 7) (8x, -0.5 folded)
                    gz = wpool.tile([128, CH], F32, tag="gz")
                    tg = wpool.tile([128, CH], F32, tag="tg")
                    nc.vector.tensor_scalar(gz[:], rgb_t[0][:, sl], w0, beta,
                                            ALU.mult, ALU.add)
                    nc.vector.tensor_scalar(tg[:], rgb_t[1][:, sl], w1, None,
                                            ALU.mult)
                    nc.vector.tensor_tensor(gz[:], gz[:], tg[:], ALU.add)
                    nc.vector.tensor_scalar(tg[:], rgb_t[2][:, sl], w2, None,
                                            ALU.mult)
                    nc.vector.tensor_tensor(gz[:], gz[:], tg[:], ALU.add)
                    nc.vector.tensor_scalar(gz[:], gz[:], 0.0, 7.0,
                                            ALU.max, ALU.min)
                    neg = wpool.tile([128, CH], F32, tag="neg")
                    nc.vector.tensor_scalar(neg[:], gz[:], -1.0, None,
                                            ALU.mult)

                    # tents T_z = relu(min(gz - z + 1, z + 1 - gz)) + families
                    fams = []   # fams[z] = (t, ut, vt, uvt)
                    for z in range(LUMA):
                        m = wpool.tile([128, CH], F32, tag="scratch")
                        nc.vector.scalar_tensor_tensor(
                            m[:], gz[:], float(-2 * z), neg[:],
                            ALU.add, ALU.min)
                        t = fpool.tile([128, CH], F32, tag=f"t{z}")
                        nc.vector.tensor_scalar(t[:], m[:], float(z + 1), 0.0,
                                                ALU.add, ALU.max)
                        ut = fpool.tile([128, CH], F32, tag=f"ut{z}")
                        nc.vector.tensor_tensor(ut[:], t[:], upl_t[:, sl],
                                                ALU.mult)
                        vt = fpool.tile([128, CH], F32, tag=f"vt{z}")
                        nc.vector.tensor_tensor(vt[:], t[:], vpl_t[:, sl],
                                                ALU.mult)
                        uvt = fpool.tile([128, CH], F32, tag=f"uvt{z}")
                        nc.vector.tensor_tensor(uvt[:], ut[:], vpl_t[:, sl],
                                                ALU.mult)
                        fams.append((t, ut, vt, uvt))

                    # contraction + affine accumulation
                    outacc = [wpool.tile([128, CH], F32, tag=f"oacc{o}",
                                         name=f"oacc{o}")
                              for o in range(NOUT)]
                    coeff = wpool.tile([128, CH], F32, tag="coeff")

                    facc = [wpool.tile([128, CH], F32, tag=f"facc{f}",
                                       name=f"facc{f}") for f in range(4)]
                    for j in range(12):
                        o, i = divmod(j, 4)
                        for f in range(4):
                            for z in range(LUMA):
                                base = half * 384 + (j * 8 + z) * 4
                                sc = vec_t[:, base + f:base + f + 1]
                                fam = fams[z][f]
                                if z == 0:
                                    nc.vector.tensor_scalar(
                                        facc[f][:], fam[:], sc, None, ALU.mult)
                                else:
                                    nc.vector.scalar_tensor_tensor(
                                        facc[f][:], fam[:], sc, facc[f][:],
                                        ALU.mult, ALU.add)
                        nc.vector.tensor_tensor(facc[0][:], facc[0][:],
                                                facc[1][:], ALU.add)
                        nc.vector.tensor_tensor(facc[2][:], facc[2][:],
                                                facc[3][:], ALU.add)
                        nc.vector.tensor_tensor(coeff[:], facc[0][:],
                                                facc[2][:], ALU.add)
                        if i < 3:
                            nc.vector.tensor_tensor(coeff[:], coeff[:],
                                                    rgb_t[i][:, sl], ALU.mult)
                        if i == 0:
                            nc.vector.tensor_copy(outacc[o][:], coeff[:])
                        else:
                            nc.vector.tensor_tensor(outacc[o][:],
                                                    outacc[o][:], coeff[:],
                                                    ALU.add)

                    for o in range(NOUT):
                        res = iopool.tile([128, CH], F16, tag=f"res{o}")
                        nc.vector.tensor_scalar(res[:], outacc[o][:],
                                                0.0, 1.0, ALU.max, ALU.min)
                        nc.sync.dma_start(OUT[o, half, :, sl], res[:])

    nc.finalize()
    return nc


# ---------------------------------------------------------------------------
# Cached PJRT runner (mirrors bass2jax.run_bass_via_pjrt, jitted once)
# ---------------------------------------------------------------------------

class _Runner:
    def __init__(self, w_guide, beta):
        nc = _build_program(w_guide, beta)
        b2j.install_neuronx_cc_hook()
        assert nc.dbg_addr is None
        pname = nc.partition_id_tensor.name if nc.partition_id_tensor else None

        in_names, out_names, out_avals = [], [], []
        for alloc in nc.m.functions[0].allocations:
            if not isinstance(alloc, mybir.MemoryLocationSet):
                continue
            name = alloc.memorylocations[0].name
            if alloc.kind == "ExternalInput":
                if name != pname:
                    in_names.append(name)
            elif alloc.kind == "ExternalOutput":
                out_names.append(name)
                out_avals.append(jax.core.ShapedArray(
                    tuple(alloc.tensor_shape), mybir.dt.np(alloc.dtype)))
        n_params = len(in_names)
        in_names = in_names + out_names
        if pname is not None:
            in_names.append(pname)
        self.in_order = in_names[:n_params]

        def _body(*args):
            operands = list(args)
            if pname is not None:
                operands.append(b2j.partition_id_tensor())
            return tuple(b2j._bass_exec_p.bind(
                *operands,
                out_avals=tuple(out_avals),
                in_names=tuple(in_names),
                out_names=tuple(out_names),
                lowering_input_output_aliases=(),
                sim_require_finite=True,
                sim_require_nnan=True,
                nc=nc,
            ))

        devices = jax.devices()[:N_CORES]
        self.mesh = Mesh(np.asarray(devices), ("core",))
        P = PartitionSpec
        self.sh = NamedSharding(self.mesh, P("core"))
        in_specs = (P("core"),) * (n_params + len(out_names))
        out_specs = (P("core"),) * len(out_names)
        self.jitted = jax.jit(
            shard_map(_body, mesh=self.mesh, in_specs=in_specs,
                      out_specs=out_specs, check_rep=False),
            keep_unused=True,
        )

        U, V = _uv_planes()
        self.upl_dev = jax.device_put(
            np.ascontiguousarray(np.broadcast_to(U, (N_CORES, 2, 128, 1024))
                                 .reshape(16, 128, 1024)), self.sh)
        self.vpl_dev = jax.device_put(
            np.ascontiguousarray(np.broadcast_to(V, (N_CORES, 128, 1024))
                                 .reshape(1024, 1024)), self.sh)
        # dummy stand-in for the ExternalOutput slot: the kernel writes every
        # output element, so no zero-init buffer needs to ship to the device.
        self.dummy = np.zeros((N_CORES, 1), np.float16)

    def __call__(self, rgb_dev, vec_concat):
        args = {"rgbq": rgb_dev, "vec": vec_concat,
                "uplanes": self.upl_dev, "vplane": self.vpl_dev}
        outs = self.jitted(*[args[n] for n in self.in_order], self.dummy)
        return np.asarray(outs[0])   # [24, 2, 128, 1024] fp16


_RUNNER_CACHE = {}


def _get_runner(w_guide, beta):
    key = (tuple(np.round(w_guide, 10)), round(beta, 10))
    if key not in _RUNNER_CACHE:
        _RUNNER_CACHE[key] = _Runner(w_guide, beta)
    return _RUNNER_CACHE[key]


# ---------------------------------------------------------------------------
# Entry point
# ---------------------------------------------------------------------------

def kernel(**inputs):
    fullres = np.asarray(inputs["image_fullres"], np.float32)
    w_guide, beta = _guide_linear_params(inputs)
    runner = _get_runner(w_guide, beta)

    # Ship the big tensor first (async) so the upload overlaps the host CNN.
    rgbq = _quadrantize_all(fullres)                     # [24,2,128,1024]
    rgb_dev = jax.device_put(rgbq, runner.sh)

    grid = _grid_from_lowres(inputs)                     # [B,12,8,16,16]
    vec = _build_vec_all(grid).reshape(8 * 128, 768)     # concat over cores

    res = runner(rgb_dev, vec)                           # [24,2,128,1024] f16
    return _unquadrantize_all(res)


# revision 3
# speedup vs baseline: 5.7690x; 1.0253x over previous
"""DeepBilateralNetCurves (HDRNet-style) Trainium2 kernel.

Split of work:
  - Host (numpy): the tiny lowres CNN (256x256 -> 12x8x16x16 bilateral grid,
    ~165 MFLOP on 1.5 MB of input), plus weight folding / layout prep.
  - Device (8 NeuronCores, Bass/Tile): the memory-bound fullres stage
    (guide map -> luma tents -> trilinear grid slice -> per-pixel affine),
    which is ~97% of the memory traffic (2x3x1024x1024 in + out).

Sharding: fullres rows are sharded 8 ways (batch b = core//4, 256 rows per
core); the tiny grid-derived constants are replicated per core.

Device layout ("quadrant layout"): for a core's [256, 1024] slice,
  partition p = xb*8 + yb*2 + hh   (xb: 16 x-blocks of 64 cols,
                                    yb: 4 local y-blocks of 64 rows,
                                    hh: which 32-row half of the y-block)
  free      f = hsub*32 + r        (hsub: row within half-block, r: col within
                                    a 32-col half of the x-block)
and two tile families per tensor: half L (w in [64xb, 64xb+32), fx = xb-1)
and half R (w in [64xb+32, 64xb+64), fx = xb).  In this layout the bilinear
corner cell indices (fy, fx) are constant per partition, so the four grid
corner combinations A, B, C, D (per output channel j and luma bin z) are
per-partition scalars, and the per-pixel trilinear slice becomes
    coeff_j = sum_z [ A*T_z + B*(u*T_z) + C*(v*T_z) + D*(u*v*T_z) ]
with T_z the luma tent weights and u, v fixed free-axis patterns.

Wall-clock structure: the axon tunnel to the remote NeuronCores has high
per-transfer latency, ~90 MB/s up, ~36 MB/s down (but close to full-duplex),
and the stock bass2jax glue re-traces and re-compiles on every
run_bass_kernel_spmd call.  So the runner here
  (a) builds + jits one shard_map executable (for a column-chunk of the
      work) once and caches it,
  (b) keeps the constant u/v planes device-resident,
  (c) ships no output donation buffers (the kernel writes every element),
  (d) returns the output as fp16 (error budget ~5e-4 << the 2e-2 gate;
      halves the slow downlink), and
  (e) splits the image into column chunks run as separate async calls so
      chunk uploads/executions overlap earlier chunks' downloads.
"""

import numpy as np

import jax

import concourse.bass as bass  # noqa: F401  (keeps bass registered)
import concourse.bacc as bacc
import concourse.bass2jax as b2j
import concourse.mybir as mybir
from concourse.tile import TileContext
from jax.experimental.shard_map import shard_map
from jax.sharding import Mesh, NamedSharding, PartitionSpec

F32 = mybir.dt.float32
F16 = mybir.dt.float16
ALU = mybir.AluOpType

LUMA, GPTS = 8, 16
NIN, NOUT = 3, 3
H, W = 1024, 1024
B = 2
N_CORES = 8
NCH = 2                      # column chunks per half; K = 2*NCH calls
CHW = 1024 // NCH            # free-dim width per chunk
HSL = 32 // NCH              # hsub values per chunk


# ---------------------------------------------------------------------------
# Host-side reference CNN (numpy float32, mirrors reference.py exactly)
# ---------------------------------------------------------------------------

def _conv(x, w, b=None, stride=1, relu=True):
    # x: [C, H, W]; w: [O, I, k, k]; cross-correlation, pad k//2
    k = w.shape[2]
    p = k // 2
    if p:
        xp = np.pad(x, ((0, 0), (p, p), (p, p)))
    else:
        xp = x
    win = np.lib.stride_tricks.sliding_window_view(xp, (k, k), axis=(1, 2))
    win = win[:, ::stride, ::stride]           # [I, Ho, Wo, k, k]
    y = np.einsum("ihwkl,oikl->ohw", win, w, optimize=True).astype(np.float32)
    if b is not None:
        y = y + b[:, None, None]
    return np.maximum(y, 0.0) if relu else y


def _grid_from_lowres(inp):
    """Returns grid [B, 12, LUMA, 16, 16] float32."""
    lows = np.asarray(inp["image_lowres"], np.float32)
    grids = []
    for bi in range(lows.shape[0]):
        x = lows[bi]
        x = _conv(x, inp["sw0"], inp["sb0"], 2)
        x = _conv(x, inp["sw1"], inp["sb1"], 2)
        x = _conv(x, inp["sw2"], inp["sb2"], 2)
        x = _conv(x, inp["sw3"], inp["sb3"], 2)          # [64,16,16]
        g = _conv(x, inp["gw0"], inp["gb0"], 2)
        g = _conv(g, inp["gw1"], inp["gb1"], 2)          # [64,4,4]
        g = g.reshape(-1)                                # [1024]
        g = np.maximum(g @ inp["fw0"].T + inp["fb0"], 0)
        g = np.maximum(g @ inp["fw1"].T + inp["fb1"], 0)
        g = g @ inp["fw2"].T + inp["fb2"]                # [64]
        loc = _conv(x, inp["lw0"], inp["lb0"], 1)
        loc = _conv(loc, inp["lw1"], None, 1, relu=False)
        fusion = np.maximum(g[:, None, None] + loc, 0)   # [64,16,16]
        co = _conv(fusion, inp["pw"], inp["pb"], 1, relu=False)  # [96,16,16]
        grid = co.reshape(LUMA, NOUT * (NIN + 1), 16, 16).transpose(1, 0, 2, 3)
        grids.append(grid.astype(np.float32))
    return np.stack(grids)                               # [B,12,8,16,16]


def _guide_linear_params(inp):
    """The guide map here is linear in rgb: verify & fold.

    guide g = clip(sum_c projw_c * pwl_c(ccm(rgb)_c) + proj_b, 0, 1),
    pwl_c(y) = sum_k slopes_ck * relu(y - shifts_ck).
    When only slope k=0 is nonzero with shift 0, and ccm output is provably
    >= 0 on [0,1]^3, pwl is linear -> g = w . rgb + beta.
    Device then computes gz = clamp(8*g - 0.5, 0, 7) (equivalent to the
    reference's clip-then-scale followed by clipped-tap accumulation).
    """
    slopes = np.asarray(inp["slopes"], np.float32).reshape(NIN, GPTS)
    shifts = np.asarray(inp["shifts"], np.float32).reshape(NIN, GPTS)
    M = np.asarray(inp["ccm_w"], np.float32).reshape(NIN, NIN)
    bc = np.asarray(inp["ccm_b"], np.float32)
    pw = np.asarray(inp["proj_w"], np.float32).reshape(NIN)
    pb = float(np.asarray(inp["proj_b"], np.float32).reshape(-1)[0])
    if not (np.all(slopes[:, 1:] == 0) and np.all(shifts[:, 0] == 0)):
        raise NotImplementedError("general piecewise-linear guide not folded")
    ymin = bc + np.minimum(M, 0).sum(axis=1)
    if not np.all(ymin >= 0):
        raise NotImplementedError("ccm output can go negative; relu not linear")
    s0 = slopes[:, 0]                                    # per-channel slope
    w = np.einsum("c,c,ci->i", pw, s0, M)
    beta = float(np.dot(pw * s0, bc) + pb)
    # fold gz = 8*g - 0.5
    return (w * 8.0).astype(np.float32), beta * 8.0 - 0.5


# ---------------------------------------------------------------------------
# Host-side layout helpers (all vectorized over the 8 cores)
# ---------------------------------------------------------------------------

def _quadrantize_chunks(fullres):
    """[B,3,1024,1024] -> [2(half), NCH(ci), 24, 128, CHW] chunk-major,
    where axis 2 is concat over cores (core = bi*4 + hblk) of per-core
    channels, axis 3 is the quadrant partition p = xb*8 + yb*2 + hh and
    axis 4 is f = hs_lo*32 + r (with hsub = ci*HSL + hs_lo)."""
    v = fullres.reshape(B, 3, 4, 4, 2, NCH, HSL, 16, 2, 32)
    #                   bi c  hblk yb hh ci  hs_lo xb half r
    v = v.transpose(8, 5, 0, 2, 1, 7, 3, 4, 6, 9)
    #               half ci bi hblk c xb yb hh hs_lo r
    return np.ascontiguousarray(v.reshape(2, NCH, 24, 128, CHW))


def _build_vec_half(grid):
    """Per-partition corner combos: [2(half), 1024(core*128+p), 384] f32,
    index (j*8+z)*4 + field, field in (A, B, C, D)."""
    hblk = np.arange(4)
    yb = np.arange(4)
    hh = np.arange(2)
    k = 8 * hblk[:, None, None] + 2 * yb[None, :, None] + hh[None, None, :]
    fy = (k - 1) // 2                                    # [4,4,2]
    cy0 = np.clip(fy, 0, 15)
    cy1 = np.clip(fy + 1, 0, 15)
    xb = np.arange(16)
    half = np.arange(2)
    fx = xb[:, None] - 1 + half[None, :]                 # [16,2]
    cx0 = np.clip(fx, 0, 15)
    cx1 = np.clip(fx + 1, 0, 15)

    def g(cy, cx):
        cyE = cy[:, :, :, None, None]                    # [4,4,2,1,1]
        cxE = cx[None, None, None, :, :]                 # [1,1,1,16,2]
        return grid[:, :, :, cyE, cxE]                   # [B,12,8,4,4,2,16,2]

    g00, g01, g10, g11 = g(cy0, cx0), g(cy0, cx1), g(cy1, cx0), g(cy1, cx1)
    F = np.stack([g00, g01 - g00, g10 - g00, g11 - g01 - g10 + g00], axis=-1)
    # F: [bi, j, z, hblk, yb, hh, xb, half, field]
    F = F.transpose(7, 0, 3, 6, 4, 5, 1, 2, 8)
    #               half bi hblk xb yb hh j z field
    return np.ascontiguousarray(F.reshape(2, 1024, 384), np.float32)


def _uv_planes():
    """U planes per half and V plane, [128, 1024] f32 each."""
    r = np.arange(32, dtype=np.float32)
    hsub = np.arange(32, dtype=np.float32)
    uL = (r + 0.5) / 64.0 + 0.5                 # half L
    uR = (r + 0.5) / 64.0                       # half R
    U = np.zeros((2, 128, 1024), np.float32)
    U[0] = np.tile(uL[None, :], (128, 32)).reshape(128, 1024)
    U[1] = np.tile(uR[None, :], (128, 32)).reshape(128, 1024)
    V = np.zeros((128, 1024), np.float32)
    vbase = (hsub + 0.5) / 64.0                 # [32]
    vplane_hh = np.repeat(vbase, 32)[None, :]   # [1, 1024] (hsub major)
    for p in range(128):
        hh = p % 2
        V[p] = vplane_hh + (0.5 if hh == 0 else 0.0)
    return U, V


# ---------------------------------------------------------------------------
# Device program: one column chunk ([3, 128, CHW] rgb -> [3, 128, CHW] out).
# All half/ci dependence enters via the data (vec / u / v planes fed in).
# ---------------------------------------------------------------------------

def _build_program(w_guide, beta):
    nc = bacc.Bacc("TRN2", target_bir_lowering=False)
    RGB = nc.dram_tensor("rgbq", [3, 128, CHW], F32, kind="ExternalInput")
    VEC = nc.dram_tensor("vec", [128, 384], F32, kind="ExternalInput")
    UPL = nc.dram_tensor("uplane", [128, CHW], F32, kind="ExternalInput")
    VPL = nc.dram_tensor("vplane", [128, CHW], F32, kind="ExternalInput")
    OUT = nc.dram_tensor("outq", [3, 128, CHW], F16, kind="ExternalOutput")

    w0, w1, w2 = (float(x) for x in w_guide)

    CH = 512  # free-dim tile

    with TileContext(nc) as tc:
        with tc.tile_pool(name="const", bufs=1) as cpool, \
             tc.tile_pool(name="io", bufs=1) as iopool, \
             tc.tile_pool(name="fam", bufs=1) as fpool, \
             tc.tile_pool(name="work", bufs=1) as wpool:

            vec_t = cpool.tile([128, 384], F32, tag="vec")
            nc.sync.dma_start(vec_t[:], VEC[:])
            vpl_t = cpool.tile([128, CHW], F32, tag="vpl")
            nc.sync.dma_start(vpl_t[:], VPL[:])
            upl_t = cpool.tile([128, CHW], F32, tag="upl")
            nc.sync.dma_start(upl_t[:], UPL[:])
            # Touch DMA'd tensors with plain copies so semaphore waits land
            # on TENSOR_COPY (ptr-scalar ISA structs have few wait slots).
            touch = cpool.tile([128, 1], F32, tag="touch")
            nc.vector.tensor_copy(touch[:], vec_t[:, 0:1])
            touchb = cpool.tile([128, 1], F32, tag="touchb")
            nc.vector.tensor_copy(touchb[:], vpl_t[:, 0:1])

            rgb_t = []
            for c in range(3):
                t = iopool.tile([128, CHW], F32, tag=f"rgb{c}")
                nc.sync.dma_start(t[:], RGB[c])
                rgb_t.append(t)

            for ci in range(CHW // CH):
                sl = slice(ci * CH, (ci + 1) * CH)
                # guide: gz = clamp(w.rgb + beta, 0, 7) (8x, -0.5 folded)
                gz = wpool.tile([128, CH], F32, tag="gz")
                tg = wpool.tile([128, CH], F32, tag="tg")
                nc.vector.tensor_scalar(gz[:], rgb_t[0][:, sl], w0, beta,
                                        ALU.mult, ALU.add)
                nc.vector.tensor_scalar(tg[:], rgb_t[1][:, sl], w1, None,
                                        ALU.mult)
                nc.vector.tensor_tensor(gz[:], gz[:], tg[:], ALU.add)
                nc.vector.tensor_scalar(tg[:], rgb_t[2][:, sl], w2, None,
                                        ALU.mult)
                nc.vector.tensor_tensor(gz[:], gz[:], tg[:], ALU.add)
                nc.vector.tensor_scalar(gz[:], gz[:], 0.0, 7.0,
                                        ALU.max, ALU.min)
                neg = wpool.tile([128, CH], F32, tag="neg")
                nc.vector.tensor_scalar(neg[:], gz[:], -1.0, None,
                                        ALU.mult)

                # tents T_z = relu(min(gz - z + 1, z + 1 - gz)) + families
                fams = []   # fams[z] = (t, ut, vt, uvt)
                for z in range(LUMA):
                    m = wpool.tile([128, CH], F32, tag="scratch")
                    nc.vector.scalar_tensor_tensor(
                        m[:], gz[:], float(-2 * z), neg[:],
                        ALU.add, ALU.min)
                    t = fpool.tile([128, CH], F32, tag=f"t{z}")
                    nc.vector.tensor_scalar(t[:], m[:], float(z + 1), 0.0,
                                            ALU.add, ALU.max)
                    ut = fpool.tile([128, CH], F32, tag=f"ut{z}")
                    nc.vector.tensor_tensor(ut[:], t[:], upl_t[:, sl],
                                            ALU.mult)
                    vt = fpool.tile([128, CH], F32, tag=f"vt{z}")
                    nc.vector.tensor_tensor(vt[:], t[:], vpl_t[:, sl],
                                            ALU.mult)
                    uvt = fpool.tile([128, CH], F32, tag=f"uvt{z}")
                    nc.vector.tensor_tensor(uvt[:], ut[:], vpl_t[:, sl],
                                            ALU.mult)
                    fams.append((t, ut, vt, uvt))

                # contraction + affine accumulation
                outacc = [wpool.tile([128, CH], F32, tag=f"oacc{o}",
                                     name=f"oacc{o}")
                          for o in range(NOUT)]
                coeff = wpool.tile([128, CH], F32, tag="coeff")

                facc = [wpool.tile([128, CH], F32, tag=f"facc{f}",
                                   name=f"facc{f}") for f in range(4)]
                for j in range(12):
                    o, i = divmod(j, 4)
                    for f in range(4):
                        for z in range(LUMA):
                            base = (j * 8 + z) * 4
                            sc = vec_t[:, base + f:base + f + 1]
                            fam = fams[z][f]
                            if z == 0:
                                nc.vector.tensor_scalar(
                                    facc[f][:], fam[:], sc, None, ALU.mult)
                            else:
                                nc.vector.scalar_tensor_tensor(
                                    facc[f][:], fam[:], sc, facc[f][:],
                                    ALU.mult, ALU.add)
                    nc.vector.tensor_tensor(facc[0][:], facc[0][:],
                                            facc[1][:], ALU.add)
                    nc.vector.tensor_tensor(facc[2][:], facc[2][:],
                                            facc[3][:], ALU.add)
                    nc.vector.tensor_tensor(coeff[:], facc[0][:],
                                            facc[2][:], ALU.add)
                    if i < 3:
                        nc.vector.tensor_tensor(coeff[:], coeff[:],
                                                rgb_t[i][:, sl], ALU.mult)
                    if i == 0:
                        nc.vector.tensor_copy(outacc[o][:], coeff[:])
                    else:
                        nc.vector.tensor_tensor(outacc[o][:],
                                                outacc[o][:], coeff[:],
                                                ALU.add)

                for o in range(NOUT):
                    res = iopool.tile([128, CH], F16, tag=f"res{o}")
                    nc.vector.tensor_scalar(res[:], outacc[o][:],
                                            0.0, 1.0, ALU.max, ALU.min)
                    nc.sync.dma_start(OUT[o, :, sl], res[:])

    nc.finalize()
    return nc


# ---------------------------------------------------------------------------
# Cached PJRT runner (mirrors bass2jax.run_bass_via_pjrt, jitted once)
# ---------------------------------------------------------------------------

class _Runner:
    def __init__(self, w_guide, beta):
        nc = _build_program(w_guide, beta)
        b2j.install_neuronx_cc_hook()
        assert nc.dbg_addr is None
        pname = nc.partition_id_tensor.name if nc.partition_id_tensor else None

        in_names, out_names, out_avals = [], [], []
        for alloc in nc.m.functions[0].allocations:
            if not isinstance(alloc, mybir.MemoryLocationSet):
                continue
            name = alloc.memorylocations[0].name
            if alloc.kind == "ExternalInput":
                if name != pname:
                    in_names.append(name)
            elif alloc.kind == "ExternalOutput":
                out_names.append(name)
                out_avals.append(jax.core.ShapedArray(
                    tuple(alloc.tensor_shape), mybir.dt.np(alloc.dtype)))
        n_params = len(in_names)
        in_names = in_names + out_names
        if pname is not None:
            in_names.append(pname)
        self.in_order = in_names[:n_params]

        def _body(*args):
            operands = list(args)
            if pname is not None:
                operands.append(b2j.partition_id_tensor())
            return tuple(b2j._bass_exec_p.bind(
                *operands,
                out_avals=tuple(out_avals),
                in_names=tuple(in_names),
                out_names=tuple(out_names),
                lowering_input_output_aliases=(),
                sim_require_finite=True,
                sim_require_nnan=True,
                nc=nc,
            ))

        devices = jax.devices()[:N_CORES]
        self.mesh = Mesh(np.asarray(devices), ("core",))
        P = PartitionSpec
        self.sh = NamedSharding(self.mesh, P("core"))
        in_specs = (P("core"),) * (n_params + len(out_names))
        out_specs = (P("core"),) * len(out_names)
        self.jitted = jax.jit(
            shard_map(_body, mesh=self.mesh, in_specs=in_specs,
                      out_specs=out_specs, check_rep=False),
            keep_unused=True,
        )

        U, V = _uv_planes()
        # uplane: per half (column pattern repeats every 32, so any CHW-wide
        # slice equals the first); vplane: per ci.
        self.upl_dev = [jax.device_put(
            np.ascontiguousarray(
                np.broadcast_to(U[h][None, :, :CHW], (N_CORES, 128, CHW))
                .reshape(N_CORES * 128, CHW)), self.sh) for h in range(2)]
        self.vpl_dev = [jax.device_put(
            np.ascontiguousarray(
                np.broadcast_to(V[None, :, ci * CHW:(ci + 1) * CHW],
                                (N_CORES, 128, CHW))
                .reshape(N_CORES * 128, CHW)), self.sh) for ci in range(NCH)]
        # dummy stand-in for the ExternalOutput slot: the kernel writes every
        # output element, so no zero-init buffer needs to ship to the device.
        self.dummy = np.zeros((N_CORES, 1), np.float16)

    def dispatch(self, rgb_dev, vec_dev, half, ci):
        args = {"rgbq": rgb_dev, "vec": vec_dev,
                "uplane": self.upl_dev[half], "vplane": self.vpl_dev[ci]}
        outs = self.jitted(*[args[n] for n in self.in_order], self.dummy)
        return outs[0]           # [24, 128, CHW] fp16, async


_RUNNER_CACHE = {}


def _get_runner(w_guide, beta):
    key = (tuple(np.round(w_guide, 10)), round(beta, 10))
    if key not in _RUNNER_CACHE:
        _RUNNER_CACHE[key] = _Runner(w_guide, beta)
    return _RUNNER_CACHE[key]


# ---------------------------------------------------------------------------
# Entry point
# ---------------------------------------------------------------------------

def kernel(**inputs):
    fullres = np.asarray(inputs["image_fullres"], np.float32)
    w_guide, beta = _guide_linear_params(inputs)
    runner = _get_runner(w_guide, beta)

    # Chunk-major relayout, then issue all uploads/execs asynchronously so the
    # tunnel pipelines: chunk k's upload overlaps chunk k-1's download.
    rgbc = _quadrantize_chunks(fullres)        # [2, NCH, 24, 128, CHW]
    chunks = [(h, ci) for h in range(2) for ci in range(NCH)]
    rgb_dev = {c: jax.device_put(rgbc[c], runner.sh) for c in chunks}

    grid = _grid_from_lowres(inputs)           # [B,12,8,16,16]
    vech = _build_vec_half(grid)               # [2, 1024, 384]
    vec_dev = [jax.device_put(vech[h], runner.sh) for h in range(2)]

    outs = {c: runner.dispatch(rgb_dev[c], vec_dev[c[0]], *c) for c in chunks}
    for c in chunks:
        outs[c].copy_to_host_async()

    final = np.empty((B, 3, 1024, 1024), np.float32)
    fview = final.reshape(B, 3, 4, 4, 2, NCH, HSL, 16, 2, 32)
    #                     bi c hblk yb hh ci hs_lo xb half r
    for (h, ci) in chunks:
        res = np.asarray(outs[(h, ci)])        # [24, 128, CHW] fp16
        v = res.reshape(B, 4, 3, 16, 4, 2, HSL, 32)
        #               bi hblk c xb yb hh hs_lo r
        fview[:, :, :, :, :, ci, :, :, h, :] = v.transpose(0, 2, 1, 4, 5, 6, 3, 7)
    return final
